# revision 52
# baseline (speedup 1.0000x reference)
"""AutoMTLSuperNet (moe_routing) Trainium2 kernel.

Strategy: batch data-parallel over 8 NeuronCores (2048 samples each, params
replicated). On-chip layout is output-channel-major ([oc, batch]) so every
layer's matmul output is directly the next layer's moving operand. All
matmuls run in bf16 with f32 PSUM accumulation; batch is processed in
chunks of 512 columns.

Engine balance (v2): candidate-mix tails use one scalar_tensor_tensor per
gelu/tanh branch; relu branches are split between ACT (relu(w*p) with the
softmax weight as activation scale) and DVE. The expert-mix broadcasts run
on the otherwise-idle GpSimd engine via partition_broadcast, with the
softmax normalization folded into the gate rows up front. Dense features
ride in the zero-padding rows of xT so the hybrid k-tile needs one DMA.
The final domain mix is computed expert-major, then transposed once.
"""

import numpy as np
import ml_dtypes

import concourse.bass as bass
import concourse.bacc as bacc
import concourse.mybir as mybir
import concourse.tile as tile
from concourse.bass_utils import run_bass_kernel_spmd

# ---- problem dims (hardcoded per contract) ----
B, F, E, D = 16384, 26, 16, 13
NE, ND, NC = 4, 3, 3
GIN = E * (F + 1) + D            # 445
H, OUT = 256, 128
N_CORES = 8
B_LOC = B // N_CORES             # 2048
NBC = 512                        # batch columns per chunk
NCHUNK = B_LOC // NBC            # 4
KSP = F * E                      # 416 flattened sparse dim
KPAD = 448                       # padded to 4 x (128,128,128,64)
BF16 = mybir.dt.bfloat16
F32 = mybir.dt.float32

AF = mybir.ActivationFunctionType
ALU = mybir.AluOpType


def _bf16(x):
    return np.asarray(x, dtype=ml_dtypes.bfloat16)


def _softmax_np(a):
    a = np.asarray(a, dtype=np.float64)
    m = a.max(axis=-1, keepdims=True)
    e = np.exp(a - m)
    return (e / e.sum(axis=-1, keepdims=True)).astype(np.float32)


def prep_shared(inputs):
    """Host prep of all parameter tensors (input-layout + parameter-only math)."""
    f32 = np.float32
    gate_w = 1.0 / (1.0 + np.exp(-inputs['feat_alpha'].astype(np.float64)))  # [NE,F]
    gate_w = gate_w.astype(f32)

    W_l0b0 = inputs['W_l0b0'].astype(f32)   # [NE,NC,GIN,H]
    W_l0b1 = inputs['W_l0b1'].astype(f32)   # [NE,NC,H,OUT]
    W_l1b0 = inputs['W_l1b0'].astype(f32)   # [NE,NC,OUT,H]
    W_l1b1 = inputs['W_l1b1'].astype(f32)   # [NE,NC,H,OUT]

    # candidate softmax weights per mixed-op layer: [4][NE,NC]
    wmix_l = [_softmax_np(inputs[k]) for k in ('a_l0b0', 'a_l0b1', 'a_l1b0', 'a_l1b1')]

    # ---- Wl0: lhsT ktiles [4,128, 3072]; col = n*768 + c*256 + h ----
    Wl0 = np.zeros((4, 128, NE * NC * H), dtype=f32)
    # sparse rows (g-folded): global row i = fe for fe in 0..415
    Wsp = np.zeros((KSP, NE, NC, H), dtype=f32)
    for n in range(NE):
        gvec = np.repeat(gate_w[n], E)                      # [416]
        Wsp[:, n] = W_l0b0[n, :, :KSP, :].transpose(1, 0, 2) * gvec[:, None, None]
    Wsp = Wsp.reshape(KSP, NE * NC * H)
    for kt in range(3):
        Wl0[kt, :, :] = Wsp[kt * 128:(kt + 1) * 128]
    # kt3 layout: [0:32]=sparse rows 384..415, [32:45]=dense, [45:64]=0,
    #             [64:128]=fm rows (64 + n*16 + e)
    Wl0[3, 0:32, :] = Wsp[384:416]
    for d in range(D):
        Wl0[3, 32 + d, :] = W_l0b0[:, :, KSP + E + d, :].reshape(-1)
    for n in range(NE):
        for e in range(E):
            Wl0[3, 64 + n * 16 + e, n * 768:(n + 1) * 768] = \
                W_l0b0[n, :, KSP + e, :].reshape(768)

    # ---- GsWg: [4,128,108]  cols 0:64 = Gs (n*16+e), gates at 64:80, 96:108 ----
    # (the 16-col gap keeps every later partition-slice offset 32-aligned)
    GsWg = np.zeros((4, 128, 108), dtype=f32)
    Gq = np.zeros((4, 128, 64), dtype=f32)
    for fe in range(KSP):
        kt, i = divmod(fe, 128)
        f_, e_ = divmod(fe, E)
        for n in range(NE):
            g = gate_w[n, f_]
            GsWg[kt, i, n * 16 + e_] = g
            Gq[kt, i, n * 16 + e_] = 0.5 * g * g   # 0.5 pre-folded
    Wg0, Wg1 = inputs['Wg0'].astype(f32), inputs['Wg1'].astype(f32)
    for i in range(KSP):
        kt, r = divmod(i, 128)
        for n in range(NE):
            for e in range(NE):
                GsWg[kt, r, 64 + e * 4 + n] = Wg0[n, i, e]
        for d in range(ND):
            for e in range(NE):
                GsWg[kt, r, 96 + d * 4 + e] = Wg1[d, i, e]
    gbias = np.zeros((44, 1), dtype=f32)
    for n in range(NE):
        for e in range(NE):
            gbias[e * 4 + n, 0] = inputs['bg0'][n, e] + inputs['beta0'][n, e]
    for d in range(ND):
        for e in range(NE):
            gbias[32 + d * 4 + e, 0] = inputs['bg1'][d, e] + inputs['beta1'][d, e]
    # sel16 [16,4]: row e*4+n -> col n  (row sums per layer-0 gate n)
    sel16 = np.zeros((16, 4), dtype=f32)
    for e in range(NE):
        for n in range(NE):
            sel16[e * 4 + n, n] = 1.0
    # r16sel [4,16]: broadcast r0 row n to rows e*4+n
    r16sel = np.zeros((4, 16), dtype=f32)
    for e in range(NE):
        for n in range(NE):
            r16sel[n, e * 4 + n] = 1.0
    # broadcast selectors: selbc[r] = e_r (x) ones128 -> lhsT picks row r of rhs
    selbc = np.zeros((16, 16, 128), dtype=f32)
    for r_ in range(16):
        selbc[r_, r_, :] = 1.0
    selbr = np.zeros((4, 4, 128), dtype=f32)
    for r_ in range(4):
        selbr[r_, r_, :] = 1.0
    # oh3 [3,12]: broadcast domain-onehot row d to rows 4d..4d+3
    oh3 = np.zeros((3, 12), dtype=f32)
    for d in range(ND):
        oh3[d, 4 * d:4 * d + 4] = 1.0
    # sel12 [12,36]: cols 0:4 sum the d-blocks per expert e; cols 32:36 all-ones
    sel12 = np.zeros((12, 36), dtype=f32)
    for d in range(ND):
        for e in range(NE):
            sel12[4 * d + e, e] = 1.0
    sel12[:, 32:36] = 1.0

    # ---- later layer weights ----
    Wb1 = np.zeros((NE, H, NC * OUT), dtype=f32)       # lhsT col = c*128+o
    for n in range(NE):
        Wb1[n] = W_l0b1[n].transpose(1, 0, 2).reshape(H, NC * OUT)
    W10 = np.zeros((NE, OUT, NC * H), dtype=f32)       # col = c*256+h
    for n in range(NE):
        W10[n] = W_l1b0[n].transpose(1, 0, 2).reshape(OUT, NC * H)
    W11 = np.zeros((NE, H, NC * OUT), dtype=f32)
    for n in range(NE):
        W11[n] = W_l1b1[n].transpose(1, 0, 2).reshape(H, NC * OUT)

    # ---- bias column tables (per-partition vectors), w-scaled for relu c=0 ----
    def bias_table(bmat, wl, n_mt):  # bmat [NE,NC,W]; returns [128, NE*NC*n_mt]
        Wd = bmat.shape[-1]
        tbl = np.zeros((128, NE * NC * (Wd // 128)), dtype=f32)
        m = 0
        for n in range(NE):
            for c in range(NC):
                for hh in range(Wd // 128):
                    v = bmat[n, c, hh * 128:(hh + 1) * 128].astype(f32)
                    if c == 0:
                        v = v * wl[n, 0]
                    tbl[:, m] = v
                    m += 1
        return tbl
    bl0b0 = bias_table(inputs['b_l0b0'], wmix_l[0], 2)   # [128,24]
    bl0b1 = bias_table(inputs['b_l0b1'], wmix_l[1], 1)   # [128,12]
    bl1b0 = bias_table(inputs['b_l1b0'], wmix_l[2], 2)   # [128,24]
    bl1b1 = bias_table(inputs['b_l1b1'], wmix_l[3], 1)   # [128,12]

    wmix = np.zeros((128, 48), dtype=f32)
    for li, wl in enumerate(wmix_l):
        for n in range(NE):
            for c in range(NC):
                wmix[:, li * 12 + n * 3 + c] = wl[n, c]

    ident = np.eye(128, dtype=f32)

    shared = {
        'Wl0': _bf16(Wl0), 'GsWg': _bf16(GsWg), 'Gq': _bf16(Gq),
        'sel16': _bf16(sel16), 'r16sel': _bf16(r16sel), 'sel12': _bf16(sel12),
        'selbc': _bf16(selbc), 'selbr': _bf16(selbr), 'oh3': _bf16(oh3),
        'Wb1': _bf16(Wb1), 'W10': _bf16(W10),
        'W11': _bf16(W11), 'gbias': gbias,
        'bl0b0': bl0b0, 'bl0b1': bl0b1, 'bl1b0': bl1b0, 'bl1b1': bl1b1,
        'wmix': wmix, 'ident': _bf16(ident),
    }
    return shared


def prep_core(inputs, r):
    """Per-core input shards (layout only)."""
    lo, hi = r * B_LOC, (r + 1) * B_LOC
    xs = inputs['sparse_embs'][lo:hi].reshape(B_LOC, KSP)      # [2048,416] f32
    xT = np.zeros((KPAD, B_LOC), dtype=ml_dtypes.bfloat16)
    xT[:KSP] = _bf16(xs.T)
    # dense features ride in the padding rows 416:429 (k-tile 3 rows 32:45)
    xT[KSP:KSP + D] = _bf16(inputs['dense_features'][lo:hi].astype(np.float32).T)
    dom = inputs['domain_ids'][lo:hi].astype(np.int64)
    dom1h = np.zeros((ND, B_LOC), dtype=ml_dtypes.bfloat16)
    for d in range(ND):
        dom1h[d] = (dom == d).astype(np.float32)
    return {'xT': xT, 'dom1h': dom1h}


def build_program(relu_dve_ok=True):
    """relu branches: ACT computes relu(w*p + w*b) with the candidate weight
    as activation scale (valid in general since softmax w>0). When biases are
    all zero (relu_dve_ok), half the relus move to DVE tensor_scalar
    (max 0, mult w) to balance the two engines."""
    nc = bacc.Bacc(trn_type="TRN2", target_bir_lowering=False, debug=False)

    # ---- DRAM I/O ----
    t_xT = nc.dram_tensor('xT', [KPAD, B_LOC], BF16, kind="ExternalInput").ap()
    t_dom1h = nc.dram_tensor('dom1h', [ND, B_LOC], BF16, kind="ExternalInput").ap()
    t_Wl0 = nc.dram_tensor('Wl0', [4, 128, 3072], BF16, kind="ExternalInput").ap()
    t_GsWg = nc.dram_tensor('GsWg', [4, 128, 108], BF16, kind="ExternalInput").ap()
    t_Gq = nc.dram_tensor('Gq', [4, 128, 64], BF16, kind="ExternalInput").ap()
    t_sel16 = nc.dram_tensor('sel16', [16, 4], BF16, kind="ExternalInput").ap()
    t_r16sel = nc.dram_tensor('r16sel', [4, 16], BF16, kind="ExternalInput").ap()
    t_sel12 = nc.dram_tensor('sel12', [12, 36], BF16, kind="ExternalInput").ap()
    t_selbc = nc.dram_tensor('selbc', [16, 16, 128], BF16, kind="ExternalInput").ap()
    t_selbr = nc.dram_tensor('selbr', [4, 4, 128], BF16, kind="ExternalInput").ap()
    t_oh3 = nc.dram_tensor('oh3', [3, 12], BF16, kind="ExternalInput").ap()
    t_Wb1 = nc.dram_tensor('Wb1', [NE, H, 384], BF16, kind="ExternalInput").ap()
    t_W10 = nc.dram_tensor('W10', [NE, OUT, 768], BF16, kind="ExternalInput").ap()
    t_W11 = nc.dram_tensor('W11', [NE, H, 384], BF16, kind="ExternalInput").ap()
    t_gbias = nc.dram_tensor('gbias', [44, 1], F32, kind="ExternalInput").ap()
    t_bl0b0 = nc.dram_tensor('bl0b0', [128, 24], F32, kind="ExternalInput").ap()
    t_bl0b1 = nc.dram_tensor('bl0b1', [128, 12], F32, kind="ExternalInput").ap()
    t_bl1b0 = nc.dram_tensor('bl1b0', [128, 24], F32, kind="ExternalInput").ap()
    t_bl1b1 = nc.dram_tensor('bl1b1', [128, 12], F32, kind="ExternalInput").ap()
    t_wmix = nc.dram_tensor('wmix', [128, 48], F32, kind="ExternalInput").ap()
    t_ident = nc.dram_tensor('ident', [128, 128], BF16, kind="ExternalInput").ap()
    t_out = nc.dram_tensor('out', [B_LOC, OUT], F32, kind="ExternalOutput").ap()

    KT_ROWS = [128, 128, 128, 64]   # xT sbuf k-tiling (kt3 loads 64 rows)
    K3 = 128

    with tile.TileContext(nc) as tc:
        with (
            tc.tile_pool(name="wpool", bufs=1) as wpool,
            tc.tile_pool(name="xpool", bufs=4) as xpool,
            tc.tile_pool(name="apool", bufs=2) as apool,
            tc.tile_pool(name="hpool", bufs=2) as hpool,
            tc.tile_pool(name="bcpool", bufs=2) as bcpool,
            tc.tile_pool(name="spool", bufs=4) as spool,
            tc.tile_pool(name="opool", bufs=2) as opool,
            tc.tile_pool(name="ps_mm", bufs=3, space="PSUM") as ps_mm,
            tc.tile_pool(name="ps_smlt", bufs=1, space="PSUM") as ps_smlt,
            tc.tile_pool(name="ps_bc", bufs=2, space="PSUM") as ps_bc,
        ):
            # ---- prologue: resident weights/constants ----
            def wtile(src_ap, shape, dtype=BF16, tag=None):
                t = wpool.tile(shape, dtype, tag=tag, name=tag)
                nc.sync.dma_start(t[:], src_ap)
                return t

            sWl0 = [wtile(t_Wl0[kt], [128, 3072], tag=f"Wl0_{kt}") for kt in range(4)]
            sGsWg = [wtile(t_GsWg[kt][:KT_ROWS[kt]], [KT_ROWS[kt], 108], tag=f"GsWg{kt}") for kt in range(4)]
            sGq = [wtile(t_Gq[kt][:KT_ROWS[kt]], [KT_ROWS[kt], 64], tag=f"Gq{kt}") for kt in range(4)]
            sSel = wtile(t_sel16, [16, 4], tag="sel16")
            sR16 = wtile(t_r16sel, [4, 16], tag="r16sel")
            sSel12 = wtile(t_sel12, [12, 36], tag="sel12")
            sSelBc = [wtile(t_selbc[r], [16, 128], tag=f"selbc{r}") for r in range(16)]
            sSelBr = [wtile(t_selbr[r], [4, 128], tag=f"selbr{r}") for r in range(4)]
            sOh3 = wtile(t_oh3, [3, 12], tag="oh3")
            sWb1 = [[wtile(t_Wb1[n][kt * 128:(kt + 1) * 128, :], [128, 384],
                           tag=f"Wb1_{n}{kt}") for kt in range(2)] for n in range(NE)]
            sW10 = [wtile(t_W10[n], [OUT, 768], tag=f"W10_{n}") for n in range(NE)]
            sW11 = [[wtile(t_W11[n][kt * 128:(kt + 1) * 128, :], [128, 384],
                           tag=f"W11_{n}{kt}") for kt in range(2)] for n in range(NE)]
            sGb = wtile(t_gbias, [44, 1], F32, tag="gbias")
            sB00 = wtile(t_bl0b0, [128, 24], F32, tag="bl0b0")
            sB01 = wtile(t_bl0b1, [128, 12], F32, tag="bl0b1")
            sB10 = wtile(t_bl1b0, [128, 24], F32, tag="bl1b0")
            sB11 = wtile(t_bl1b1, [128, 12], F32, tag="bl1b1")
            sWmix = wtile(t_wmix, [128, 48], F32, tag="wmix")
            sId = wtile(t_ident, [128, 128], tag="ident")

            # per-chunk state carried between phases
            xk = [None] * NCHUNK
            gexp = [None] * NCHUNK
            oht = [None] * NCHUNK
            e0n = [None] * NCHUNK
            wn = [None] * NCHUNK
            hA = [None] * NCHUNK
            hB = [None] * NCHUNK
            bcb = [None] * NCHUNK
            wnb = [None] * NCHUNK
            mixed = [None] * NCHUNK
            hC = [None] * NCHUNK
            h2 = [None] * NCHUNK

            import itertools
            uid = itertools.count()

            def mixed_op_tail(plist, out_t, bias_cols, w_cols, relu_on_act, tmp_tag):
                """candidate-mix tail: plist = [p_relu, p_gelu, p_tanh] psum tiles.
                out = w0*relu(p0+b0) + w1*gelu(p1+b1) + w2*tanh(p2+b2)."""
                b0, b1, b2 = bias_cols
                w0, w1, w2 = w_cols
                if relu_on_act or not relu_dve_ok:
                    # b0 is pre-scaled by w0 on host; relu(w*p + w*b) = w*relu(p+b)
                    nc.scalar.activation(out_t[:], plist[0][:], AF.Relu,
                                         bias=b0, scale=w0)
                else:
                    nc.vector.tensor_scalar(out_t[:], plist[0][:], 0.0, w0,
                                            ALU.max, ALU.mult)
                t1 = apool.tile([128, NBC], BF16, tag="tg" + tmp_tag,
                                name=f"tg{tmp_tag}_{next(uid)}")
                nc.scalar.activation(t1[:], plist[1][:], AF.Gelu_apprx_tanh, bias=b1)
                nc.vector.scalar_tensor_tensor(out_t[:], t1[:], w1, out_t[:],
                                               ALU.mult, ALU.add)
                t2 = apool.tile([128, NBC], BF16, tag="tt" + tmp_tag,
                                name=f"tt{tmp_tag}_{next(uid)}")
                nc.scalar.activation(t2[:], plist[2][:], AF.Tanh, bias=b2)
                nc.vector.scalar_tensor_tensor(out_t[:], t2[:], w2, out_t[:],
                                               ALU.mult, ALU.add)

            # ============ P0: loads, squares, fm, gates, softmax prep ============
            def phase0(ch):
                cc = ch * NBC
                xk[ch] = []
                for kt in range(4):
                    rows = 128 if kt < 3 else K3
                    t = xpool.tile([rows, NBC], BF16, tag=f"x{kt}", name=f"x{kt}_{ch}")
                    nc.sync.dma_start(t[0:KT_ROWS[kt], :],
                                      t_xT[kt * 128: kt * 128 + KT_ROWS[kt], cc:cc + NBC])
                    xk[ch].append(t)
                hyb = xk[ch][3]          # rows 0:64 from DRAM, 64:128 = fm below
                oh = xpool.tile([ND, NBC], BF16, tag="oh", name=f"oh_{ch}")
                nc.sync.dma_start(oh[:], t_dom1h[:, cc:cc + NBC])

                xq = []
                for kt in range(4):
                    t = xpool.tile([KT_ROWS[kt], NBC], BF16, tag=f"xq{kt}",
                                   name=f"xq{kt}_{ch}", bufs=2)
                    src = xk[ch][kt][0:KT_ROWS[kt], :]
                    nc.vector.tensor_tensor(t[:], src, src, ALU.mult)
                    xq.append(t)

                # s (rows 0:64) | g0 (64:80) | gap | g1 (96:108)
                sg_ps = ps_smlt.tile([128, NBC], F32, tag="smlt", name=f"sg_{ch}",
                                     bufs=1)
                for kt in range(4):
                    nc.tensor.matmul(sg_ps[0:108, :], sGsWg[kt][:],
                                     xk[ch][kt][0:KT_ROWS[kt], :],
                                     start=(kt == 0), stop=(kt == 3))
                q_ps = ps_smlt.tile([64, NBC], F32, tag="smltq", name=f"q_{ch}",
                                    bufs=1)
                for kt in range(4):
                    nc.tensor.matmul(q_ps[:], sGq[kt][:], xq[kt][:],
                                     start=(kt == 0), stop=(kt == 3))
                ssq = spool.tile([64, NBC], F32, tag="ssq", name=f"ssq_{ch}", bufs=2)
                nc.scalar.activation(ssq[:], sg_ps[0:64, :], AF.Square,
                                     scale=float(np.sqrt(0.5)))
                nc.vector.tensor_tensor(hyb[64:128, :], ssq[:], q_ps[:],
                                        ALU.subtract)

                ge = spool.tile([44, NBC], BF16, tag="gexp", name=f"gexp_{ch}")
                nc.scalar.activation(ge[0:16, :], sg_ps[64:80, :], AF.Exp,
                                     bias=sGb[0:16, 0:1])
                nc.scalar.activation(ge[32:44, :], sg_ps[96:108, :], AF.Exp,
                                     bias=sGb[32:44, 0:1])
                gexp[ch] = ge
                oht[ch] = oh

            # ============ P0b: softmax normalize chains (emitted mid-P1) ============
            def phase0b(ch):
                ge = gexp[ch]
                # layer-0 gate softmax normalization: e0n = e0 / rowsum_n
                s_ps = ps_smlt.tile([16, NBC], F32, tag="smlts", name=f"s0_{ch}",
                                    bufs=1)
                nc.tensor.matmul(s_ps[0:4, :], sSel[:], ge[0:16, :],
                                 start=True, stop=True)
                r = spool.tile([4, NBC], BF16, tag="r0", name=f"r0_{ch}", bufs=2)
                with nc.allow_low_precision("softmax recip feeds bf16 mix"):
                    nc.vector.reciprocal(r[:], s_ps[0:4, :])
                r16_ps = ps_smlt.tile([16, NBC], F32, tag="smlts", name=f"r16_{ch}",
                                      bufs=1)
                nc.tensor.matmul(r16_ps[:], sR16[:], r[:], start=True, stop=True)
                en = spool.tile([16, NBC], BF16, tag="e0n", name=f"e0n_{ch}")
                nc.vector.tensor_tensor(en[:], ge[0:16, :], r16_ps[:], ALU.mult)
                e0n[ch] = en

                # domain gate weights: mask by domain onehot, then select + sum
                ohb_ps = ps_bc.tile([12, NBC], F32, tag="bcp", name=f"ohb_{ch}")
                nc.tensor.matmul(ohb_ps[:], sOh3[:], oht[ch][:], start=True, stop=True)
                ws12 = spool.tile([12, NBC], BF16, tag="ws", name=f"ws_{ch}", bufs=2)
                nc.vector.tensor_tensor(ws12[:], ge[32:44, :], ohb_ps[:], ALU.mult)
                # sel12: rows 0:4 = per-expert selected exp-logit, 32:36 = sum
                sw_ps = ps_smlt.tile([36, NBC], F32, tag="smltq", name=f"sw_{ch}",
                                     bufs=1)
                nc.tensor.matmul(sw_ps[:], sSel12[:], ws12[:], start=True, stop=True)
                rw = spool.tile([NE, NBC], BF16, tag="rw", name=f"rw_{ch}", bufs=2)
                with nc.allow_low_precision("domain softmax recip feeds bf16 mix"):
                    nc.vector.reciprocal(rw[:], sw_ps[32:36, :])
                wnt = spool.tile([NE, NBC], BF16, tag="wn", name=f"wn_{ch}")
                nc.vector.tensor_tensor(wnt[:], sw_ps[0:4, :], rw[:], ALU.mult)
                wn[ch] = wnt



            # ============ P1: L0b0 + mix -> hA ; L0b1 + mix -> hB ============
            def phase1(ch):
                hA[ch] = {}
                for n in range(NE):
                    if n == 1:
                        phase0b(ch)
                    for hh in range(2):
                        out_t = hpool.tile([128, NBC], BF16, tag=f"hA{n}{hh}",
                                           name=f"hA{n}{hh}_{ch}", bufs=1)
                        hA[ch][(n, hh)] = out_t
                        ps = []
                        for c in range(NC):
                            m = n * 6 + c * 2 + hh
                            p = ps_mm.tile([128, NBC], F32, tag="pmm", name=f"pA{m}_{ch}")
                            for kt in range(3):
                                nc.tensor.matmul(p[:], sWl0[kt][:, m * 128:(m + 1) * 128],
                                                 xk[ch][kt][:], start=(kt == 0), stop=False)
                            nc.tensor.matmul(p[:], sWl0[3][0:K3, m * 128:(m + 1) * 128],
                                             xk[ch][3][:], start=False, stop=True)
                            ps.append(p)
                        mcols = [n * 6 + c * 2 + hh for c in range(NC)]
                        mixed_op_tail(
                            ps, out_t,
                            [sB00[:, m:m + 1] for m in mcols],
                            [sWmix[:, n * 3 + c:n * 3 + c + 1] for c in range(NC)],
                            relu_on_act=((n + hh) % 2 == 0), tmp_tag=f"A{hh}")
                hB[ch] = {}
                for n in range(NE):
                    hb = hpool.tile([128, NBC], BF16, tag=f"hB{n}", name=f"hB{n}_{ch}")
                    hB[ch][n] = hb
                    ps = []
                    for c in range(NC):
                        p = ps_mm.tile([128, NBC], F32, tag="pmm", name=f"pB{n}{c}_{ch}")
                        for kt in range(2):
                            nc.tensor.matmul(p[:], sWb1[n][kt][:, c * 128:(c + 1) * 128],
                                             hA[ch][(n, kt)][:], start=(kt == 0), stop=(kt == 1))
                        ps.append(p)
                    mcols = [n * 3 + c for c in range(NC)]
                    mixed_op_tail(
                        ps, hb,
                        [sB01[:, m:m + 1] for m in mcols],
                        [sWmix[:, 12 + m:12 + m + 1] for m in mcols],
                        relu_on_act=(n % 2 == 0), tmp_tag="B")

            # ============ P2: expert mixing 0 (normalized bcast rows) ============
            # broadcast rows via PE into packed bf16 psum pairs; DVE multiplies
            # straight from PSUM (2-byte operands keep the fast DVE mode).
            def phase2(ch):
                mixed[ch] = {}
                for n in range(NE):
                    bcv = []
                    for e in range(NE):
                        bp = ps_bc.tile([128, NBC], F32, tag="bcp",
                                        name=f"bc{n}{e}_{ch}")
                        nc.tensor.matmul(bp[:], sSelBc[e * 4 + n][:], e0n[ch][:],
                                         start=True, stop=True)
                        bcv.append(bp)
                    acc = hpool.tile([128, NBC], BF16, tag=f"mix{n}", name=f"mix{n}_{ch}",
                                     bufs=1)
                    nc.vector.tensor_tensor(acc[:], hB[ch][0][:], bcv[0][:], ALU.mult)
                    for e in range(1, NE):
                        t2 = bcpool.tile([128, NBC], BF16, tag="mixt",
                                         name=f"mixt{n}{e}_{ch}")
                        nc.vector.tensor_tensor(t2[:], hB[ch][e][:], bcv[e][:], ALU.mult)
                        nc.vector.tensor_tensor(acc[:], acc[:], t2[:], ALU.add)
                    mixed[ch][n] = acc

            # ============ P3: L1b0 + mix -> hC ; L1b1 + mix -> h2 ============
            def phase3(ch):
                hC[ch] = {}
                for n in range(NE):
                    for hh in range(2):
                        out_t = hpool.tile([128, NBC], BF16, tag=f"hC{n}{hh}",
                                           name=f"hC{n}{hh}_{ch}", bufs=1)
                        hC[ch][(n, hh)] = out_t
                        ps = []
                        for c in range(NC):
                            mt = c * 2 + hh
                            p = ps_mm.tile([128, NBC], F32, tag="pmm",
                                           name=f"pC{n}{c}{hh}_{ch}")
                            nc.tensor.matmul(p[:], sW10[n][:, mt * 128:(mt + 1) * 128],
                                             mixed[ch][n][:], start=True, stop=True)
                            ps.append(p)
                        mcols = [n * 6 + c * 2 + hh for c in range(NC)]
                        mixed_op_tail(
                            ps, out_t,
                            [sB10[:, m:m + 1] for m in mcols],
                            [sWmix[:, 24 + n * 3 + c:24 + n * 3 + c + 1] for c in range(NC)],
                            relu_on_act=((n + hh) % 2 == 1), tmp_tag=f"C{hh}")
                h2[ch] = {}
                for n in range(NE):
                    hb = hpool.tile([128, NBC], BF16, tag=f"h2{n}", name=f"h2{n}_{ch}",
                                    bufs=1)
                    h2[ch][n] = hb
                    ps = []
                    for c in range(NC):
                        p = ps_mm.tile([128, NBC], F32, tag="pmm", name=f"pD{n}{c}_{ch}")
                        for kt in range(2):
                            nc.tensor.matmul(p[:], sW11[n][kt][:, c * 128:(c + 1) * 128],
                                             hC[ch][(n, kt)][:], start=(kt == 0), stop=(kt == 1))
                        ps.append(p)
                    mcols = [n * 3 + c for c in range(NC)]
                    mixed_op_tail(
                        ps, hb,
                        [sB11[:, m:m + 1] for m in mcols],
                        [sWmix[:, 36 + m:36 + m + 1] for m in mcols],
                        relu_on_act=(n % 2 == 1), tmp_tag="Dx")

            # ============ P4: domain mix (expert-major) + transpose + out ============
            def phase4(ch):
                cc = ch * NBC
                em = opool.tile([128, NBC], BF16, tag="em", name=f"em_{ch}")
                wb = []
                for e in range(2):
                    bp = ps_bc.tile([128, NBC], F32, tag="bcp", name=f"wb{e}_{ch}")
                    nc.tensor.matmul(bp[:], sSelBr[e][:], wn[ch][:],
                                     start=True, stop=True)
                    wb.append(bp)
                nc.vector.tensor_tensor(em[:], h2[ch][0][:], wb[0][:], ALU.mult)
                for e in range(1, NE):
                    if e + 1 < NE:
                        bp = ps_bc.tile([128, NBC], F32, tag="bcp", name=f"wb{e + 1}_{ch}")
                        nc.tensor.matmul(bp[:], sSelBr[e + 1][:], wn[ch][:],
                                         start=True, stop=True)
                        wb.append(bp)
                    t2 = opool.tile([128, NBC], BF16, tag="emt", name=f"emt{e}_{ch}")
                    nc.vector.tensor_tensor(t2[:], h2[ch][e][:], wb[e][:], ALU.mult)
                    nc.vector.tensor_tensor(em[:], em[:], t2[:], ALU.add)
                tp = ps_smlt.tile([128, NBC], BF16, tag="smlts", name=f"otp_{ch}",
                                  bufs=1)
                for bt in range(4):
                    nc.tensor.transpose(tp[:, bt * 128:(bt + 1) * 128],
                                        em[:, bt * 128:(bt + 1) * 128], sId[:])
                ote = opool.tile([128, NBC], F32, tag="ote", name=f"ote_{ch}")
                nc.scalar.copy(ote[:], tp[:])
                for bt in range(4):
                    nc.sync.dma_start(t_out[cc + bt * 128: cc + (bt + 1) * 128, :],
                                      ote[:, bt * 128:(bt + 1) * 128])

            # ---- emission schedule: P0 all, then rounds with one-chunk lag ----
            for ch in range(NCHUNK):
                phase0(ch)
            for ch in range(NCHUNK):
                phase1(ch)
                if ch > 0:
                    phase2(ch - 1)
                    phase3(ch - 1)
                    phase4(ch - 1)
            phase2(NCHUNK - 1)
            phase3(NCHUNK - 1)
            phase4(NCHUNK - 1)
    nc.compile()
    return nc


_CACHE = {}


def kernel(**inputs):
    shared = prep_shared(inputs)
    in_maps = []
    for r in range(N_CORES):
        m = dict(shared)
        m.update(prep_core(inputs, r))
        in_maps.append(m)
    relu_dve_ok = (np.abs(inputs['b_l0b0']).max() == 0.0
                   and np.abs(inputs['b_l1b0']).max() == 0.0
                   and np.abs(inputs['b_l0b1']).max() == 0.0
                   and np.abs(inputs['b_l1b1']).max() == 0.0)
    key = ('nc', bool(relu_dve_ok))
    if key not in _CACHE:
        _CACHE[key] = build_program(relu_dve_ok=relu_dve_ok)
        _CACHE['nc'] = _CACHE[key]
    nc = _CACHE[key]
    res = run_bass_kernel_spmd(nc, in_maps, core_ids=list(range(N_CORES)))
    out = np.concatenate([res.results[r]['out'] for r in range(N_CORES)], axis=0)
    return out.astype(np.float32)
